# revision 22
# baseline (speedup 1.0000x reference)
"""PixelPrototypeDistanceLoss on 8 Trainium2 NeuronCores.

Math: for each pixel p with label lb_p != 19:
    logit_p = emb_pixel_p . segment_queue[lb_p]
    loss = mean((1 - logit_p)^2)  over valid pixels

Trick: with onehot[c,p] = (lb_p == c) for c in [0,19), ignored pixels match
nothing, so
    sum_p valid*(1-logit)^2 = count - 2*S1 + S2
with S1 = sum(sim*onehot), S2 = sum((sim*onehot)^2); count comes from the
host (numpy popcount over labels), so the kernel only produces S1/S2.

Sharding: batch dim across the 8 cores (one image each).  Per core:
  sim tiles [19, cg] computed as QT.T @ X with X = emb[b] reshaped [256, N]
  (already channels-first, no transpose needed).  Four pixel-stacks at PE
  col-group offsets 0/32/64/96 so the DVE sees [128, cg] blocks.  QT is
  zero-padded to 32 columns so every PSUM row is written.
  scalar_tensor_tensor fuses onehot*sim with the row-sum for S1; S2 comes
  from Square on ScalarE for the big mid-stream groups and from a second
  DVE STT for the small head/tail groups, so the post-stream drain never
  waits on a long ScalarE chain.
Pipelining: emb cast to fp8-e4m3 on host and packed into one DRAM tensor
whose first 64 columns are the QT prototypes -- the first x DMA delivers
both, so the PE unblocks as early as possible with zero extra DMA issues.
All DMAs ride the single sync HWDGE ring (a second ring interleaves
packets and starves/slows the stream); the onehot rides second, right
after the small leading x tile.  Tapered trailing tiles shorten the
serial drain after the last HBM byte.  A PE ones-reduce (reusing the
framework's f32 ones const) packs the 16 partial accumulators into one
single-descriptor [1,16] output DMA.
Host: sums the partials in f64 and forms (count - 2*S1 + S2)/count.
"""

import numpy as np
import ml_dtypes

import concourse.bacc as bacc
import concourse.mybir as mybir
from concourse.tile import TileContext
from concourse import bass_utils

# Problem dims (hardcoded per harness contract).
B, D, H, W, C = 8, 256, 128, 128, 19
NPX = H * W          # 16384 pixels per core (one batch image)
NCORES = 8
IGNORE = 19

CP = 32              # padded class count (PE tile_position granularity)
QTC = 256            # qt prefix bytes in the x tensor (padded to the 256 B
                     # DRAM page so every x row stays page-aligned)

# x DMA tiles (pixel counts): small first tile -> early pipeline start,
# tapered tail -> short serial drain.  One compute group per tile.
XTILES = [2048, 4096, 4096, 2048, 2048, 1024, 512, 512]
# S2 square batches over the resident t1 buffer: [start_group, end_group).
# The last chunk runs on the DVE; earlier ones on ScalarE, sized so each is
# ready before the next (no ScalarE chaining into the drain).
SQCH = [(0, 2), (2, 3), (3, 5), (5, 7), (7, 8)]
assert sum(XTILES) == NPX
NG = len(XTILES)
CGS = [n // 4 for n in XTILES]
OFFS = np.cumsum([0] + CGS).tolist()
LBB_COLS = NPX // 4  # total onehot columns

EMB_DT = mybir.dt.float8e4
EMB_NP = ml_dtypes.float8_e4m3

_CACHE = {}


def _blocks(cg):
    """Split cg into matmul moving-dim blocks of <=512 (PSUM bank limit)."""
    out = []
    o = 0
    while o < cg:
        b = min(512, cg - o)
        out.append((o, b))
        o += b
    return out


def _build():
    if "nc" in _CACHE:
        return _CACHE["nc"]
    nc = bacc.Bacc(
        "TRN2",
        target_bir_lowering=False,
        debug=False,
        enable_asserts=False,
    )
    # x packed on host as [128, QTC + 2*NPX]: cols [0, QTC) hold the QT
    # prototypes (col 32k+c = QT[128k+p, c], classes >= 19 zero); group
    # g's pixel block at cols [QTC + 2*base_g, ...), chunk k at
    # block-local cols [k*n, (k+1)*n)
    x_t = nc.dram_tensor("x", [128, QTC + 2 * NPX], EMB_DT,
                         kind="ExternalInput")
    # onehot u8 (group-blocked)
    meta_t = nc.dram_tensor("meta", [128, LBB_COLS], mybir.dt.uint8,
                            kind="ExternalInput")
    OUTC = 16  # pad to 64 B so the single output descriptor is aligned
    # acc cols: 0..NG-2 = s1 of g0..g6, NG-1..NG+1 = sqA/sqB/sqC,
    #           NG+2 = s1 of g7, NG+3 = sqD  (MM1 reduces 0..NG+1, MM2 rest)
    out_t = nc.dram_tensor("out", [1, OUTC], mybir.dt.float32,
                           kind="ExternalOutput")

    x = x_t.ap()
    AO = mybir.AluOpType

    with TileContext(nc) as tc:
        with (
            tc.tile_pool(name="xp", bufs=1) as xpool,
            tc.tile_pool(name="mp", bufs=1) as mpool,
            tc.tile_pool(name="scr", bufs=4) as spool,
            tc.tile_pool(name="jnk", bufs=2) as jpool,
            tc.tile_pool(name="acc", bufs=1) as apool,
            tc.tile_pool(name="ps", bufs=3, space="PSUM") as pspool,
            tc.tile_pool(name="rps", bufs=1, space="PSUM") as rpool,
        ):
            # single sync-ring stream: x0 (with qt at its head), onehot,
            # then the remaining x tiles
            xt = {}
            metat = None
            base = 0
            for g, n in enumerate(XTILES):
                ext = QTC if g == 0 else 0
                t = xpool.tile([128, ext + 2 * n], EMB_DT, tag=f"xg{g}")
                nc.sync.dma_start(
                    t[:, :],
                    x[:, QTC + 2 * base - ext:QTC + 2 * base + 2 * n])
                xt[g] = t
                if g == 0:
                    metat = mpool.tile([128, LBB_COLS], mybir.dt.uint8)
                    nc.sync.dma_start(metat[:, :], meta_t.ap())
                base += n

            lbbt = metat[:, :]
            qt_sb = xt[0][:, 0:2 * CP]

            acc = apool.tile([128, NG + 5], mybir.dt.float32)
            rps = rpool.tile([128, NG + 5], mybir.dt.float32, tag="rps")
            t1all = mpool.tile([128, LBB_COLS], mybir.dt.bfloat16)

            for g, n in enumerate(XTILES):
                cg = CGS[g]
                off = OFFS[g]
                ext = QTC if g == 0 else 0
                ps = pspool.tile([128, cg], mybir.dt.float32, tag="ps")
                for s in range(4):
                    for (mo, fb) in _blocks(cg):
                        for k in range(2):
                            col = ext + k * n + s * cg + mo
                            nc.tensor.matmul(
                                out=ps[CP * s:CP * (s + 1), mo:mo + fb],
                                lhsT=qt_sb[:, k * CP:(k + 1) * CP],
                                rhs=xt[g][:, col:col + fb],
                                start=(k == 0), stop=(k == 1),
                                tile_position=(0, CP * s))

                # t1 = onehot * sim into the resident buffer;
                # acc s1 col = row-sum(t1).  S2 squares are deferred and
                # batched (t1 is SBUF-resident), so ScalarE's big squares
                # run in mid-stream slack and never clog the drain.
                s1col = NG + 2 if g == NG - 1 else g
                nc.vector.scalar_tensor_tensor(
                    out=t1all[:, off:off + cg], in0=lbbt[:, off:off + cg],
                    scalar=1.0, in1=ps[:, :], op0=AO.mult, op1=AO.mult,
                    accum_out=acc[:, s1col:s1col + 1])
                for ci, (ga, gb) in enumerate(SQCH):
                    if g == gb - 1:
                        a, bcol = OFFS[ga], OFFS[gb]
                        jk = jpool.tile([128, bcol - a], mybir.dt.bfloat16,
                                        tag=f"jk{ci}")
                        if ci < len(SQCH) - 2:
                            # mid-stream square batches on ScalarE
                            nc.scalar.activation(
                                jk[:, :], t1all[:, a:bcol],
                                mybir.ActivationFunctionType.Square,
                                accum_out=acc[:, NG - 1 + ci:NG + ci])
                        else:
                            # small final chunk on the DVE (cheap acc read)
                            dvecol = NG + 3 + (ci - (len(SQCH) - 2))
                            nc.vector.scalar_tensor_tensor(
                                out=jk[:, :], in0=t1all[:, a:bcol],
                                scalar=1.0, in1=t1all[:, a:bcol],
                                op0=AO.mult, op1=AO.mult,
                                accum_out=acc[:, dvecol:dvecol + 1])
                if g == NG - 2:
                    # reduce everything except the last group's s1 and the
                    # final square chunk -- off the critical path
                    ones = nc.const_aps.aps[(mybir.dt.float32, 1.0)]
                    nc.tensor.matmul(
                        out=rps[0:1, 0:NG + 2], lhsT=ones,
                        rhs=acc[:, 0:NG + 2], start=True, stop=True,
                        tile_position=(0, 0))

            # fold the last group's pair into the same PSUM tile, then one
            # single-descriptor [1, 16] DMA
            ones = nc.const_aps.aps[(mybir.dt.float32, 1.0)]
            nc.tensor.matmul(out=rps[0:1, NG + 2:NG + 5], lhsT=ones,
                             rhs=acc[:, NG + 2:NG + 5], start=True,
                             stop=True, tile_position=(0, 0))
            res = apool.tile([1, 16], mybir.dt.float32)
            nc.vector.tensor_copy(res[:, 0:NG + 5], rps[0:1, :])
            nc.vector.memset(res[:, NG + 5:16], 0.0)
            nc.sync.dma_start(out_t.ap(), res[:, :])

    nc.compile()
    _CACHE["nc"] = nc
    return nc


def _prep_in_maps(emb, lb, segment_queue):
    emb = np.asarray(emb)
    lb = np.asarray(lb)
    q = np.asarray(segment_queue, dtype=np.float32)

    qt = np.zeros((D, CP), np.float32)
    qt[:, :C] = q.T
    # pack [2,128,CP] -> [128, 2*CP]: col 32k+c = QT[128k+p, c]
    qt = np.ascontiguousarray(
        qt.reshape(2, 128, CP).transpose(1, 0, 2).reshape(128, 2 * CP)
        .astype(EMB_NP))

    cls_pat = np.where(np.arange(CP) < C, np.arange(CP), -1)  # [32]

    in_maps = []
    for b in range(B):
        x8 = emb[b].reshape(2, 128, NPX).astype(EMB_NP)
        # pack per DMA tile: xb[p, QTC + 2*base + k*n + j] = x8[k, p, base+j]
        xb = np.empty((128, QTC + 2 * NPX), EMB_NP)
        xb[:, :2 * CP] = qt
        xb[:, 2 * CP:QTC] = 0
        base = 0
        for n in XTILES:
            blk = x8[:, :, base:base + n]            # [2, 128, n]
            xb[:, QTC + 2 * base:QTC + 2 * base + 2 * n] = (
                blk.transpose(1, 0, 2).reshape(128, 2 * n))
            base += n
        lbf = lb[b].reshape(-1).astype(np.float32)
        # onehot[32*s + c, off_g + j] = (lb[base_g + s*cg + j] == c)
        segs = []
        base = 0
        for g, n in enumerate(XTILES):
            cg = CGS[g]
            seg = lbf[base:base + n].reshape(4, 1, cg)
            segs.append((seg == cls_pat[None, :, None]).reshape(128, cg))
            base += n
        meta = np.ascontiguousarray(
            np.concatenate(segs, axis=1).astype(np.uint8))
        in_maps.append({
            "x": xb,
            "meta": meta,
        })
    return in_maps


def _reduce_outputs(results, count):
    s1 = 0.0
    s2 = 0.0
    for r in results:
        o = np.asarray(r["out"], dtype=np.float64)
        s1 += o[0, 0:NG - 1].sum() + o[0, NG + 2]
        s2 += (o[0, NG - 1] + o[0, NG] + o[0, NG + 1] + o[0, NG + 3]
               + o[0, NG + 4])
    num = count - 2.0 * s1 + s2
    return np.float32(num / count)


def run_on_cores(inputs, **kwargs):
    """Run the bass kernel on cores 0-7; returns (loss, BassKernelResults).

    The device occasionally reports a transient NRT_EXEC_UNIT_UNRECOVERABLE
    on a run that succeeds on immediate retry; retry a couple of times.
    """
    nc = _build()
    in_maps = _prep_in_maps(**inputs)
    count = float(np.count_nonzero(np.asarray(inputs["lb"]) != IGNORE))
    last_err = None
    for _ in range(3):
        try:
            res = bass_utils.run_bass_kernel_spmd(
                nc, in_maps, core_ids=list(range(NCORES)), **kwargs)
            return _reduce_outputs(res.results, count), res
        except Exception as e:  # transient device wedge -> retry
            last_err = e
    raise last_err


def kernel(emb, lb, segment_queue):
    loss, _ = run_on_cores({"emb": emb, "lb": lb, "segment_queue": segment_queue})
    return loss


# revision 23
# speedup vs baseline: 1.0828x; 1.0828x over previous
"""PixelPrototypeDistanceLoss on 8 Trainium2 NeuronCores.

Math: for each pixel p with label lb_p != 19:
    logit_p = emb_pixel_p . segment_queue[lb_p]
    loss = mean((1 - logit_p)^2)  over valid pixels

Trick: with onehot[c,p] = (lb_p == c) for c in [0,19), ignored pixels match
nothing, so
    sum_p valid*(1-logit)^2 = count - 2*S1 + S2
with S1 = sum(sim*onehot), S2 = sum((sim*onehot)^2); count comes from the
host (numpy popcount over labels), so the kernel only produces S1/S2.

Sharding: batch dim across the 8 cores (one image each).  Per core:
  sim tiles [19, cg] computed as QT.T @ X with X = emb[b] reshaped [256, N]
  (already channels-first, no transpose needed).  Four pixel-stacks at PE
  col-group offsets 0/32/64/96 so the DVE sees [128, cg] blocks.  QT is
  zero-padded to 32 columns so every PSUM row is written.
  scalar_tensor_tensor fuses onehot*sim with the row-sum for S1; S2 comes
  from Square on ScalarE for the big mid-stream groups and from a second
  DVE STT for the small head/tail groups, so the post-stream drain never
  waits on a long ScalarE chain.
Pipelining: emb cast to fp8-e4m3 on host and packed into one DRAM tensor
whose first 64 columns are the QT prototypes -- the first x DMA delivers
both, so the PE unblocks as early as possible with zero extra DMA issues.
All DMAs ride the single sync HWDGE ring (a second ring interleaves
packets and starves/slows the stream); the onehot rides second, right
after the small leading x tile.  Tapered trailing tiles shorten the
serial drain after the last HBM byte.  A PE ones-reduce (reusing the
framework's f32 ones const) packs the 16 partial accumulators into one
single-descriptor [1,16] output DMA.
Host: sums the partials in f64 and forms (count - 2*S1 + S2)/count.
"""

import numpy as np
import ml_dtypes

import concourse.bacc as bacc
import concourse.mybir as mybir
from concourse.tile import TileContext
from concourse import bass_utils

# Problem dims (hardcoded per harness contract).
B, D, H, W, C = 8, 256, 128, 128, 19
NPX = H * W          # 16384 pixels per core (one batch image)
NCORES = 8
IGNORE = 19

CP = 32              # padded class count (PE tile_position granularity)
QTC = 256            # qt prefix bytes in the x tensor (padded to the 256 B
                     # DRAM page so every x row stays page-aligned)

# x DMA tiles (pixel counts): small first tile -> early pipeline start,
# tapered tail -> short serial drain.  One compute group per tile.
XTILES = [2048, 4096, 4096, 2048, 2048, 1024, 512, 512]
# S2 square batches over the resident t1 buffer: [start_group, end_group).
# The last chunk runs on the DVE; earlier ones on ScalarE, sized so each is
# ready before the next (no ScalarE chaining into the drain).
SQCH = [(0, 2), (2, 3), (3, 5), (5, 8)]
assert sum(XTILES) == NPX
NG = len(XTILES)
CGS = [n // 4 for n in XTILES]
OFFS = np.cumsum([0] + CGS).tolist()
LBB_COLS = NPX // 4  # total onehot columns

EMB_DT = mybir.dt.float8e4
EMB_NP = ml_dtypes.float8_e4m3

_CACHE = {}


def _blocks(cg):
    """Split cg into matmul moving-dim blocks of <=512 (PSUM bank limit)."""
    out = []
    o = 0
    while o < cg:
        b = min(512, cg - o)
        out.append((o, b))
        o += b
    return out


def _build():
    if "nc" in _CACHE:
        return _CACHE["nc"]
    nc = bacc.Bacc(
        "TRN2",
        target_bir_lowering=False,
        debug=False,
        enable_asserts=False,
    )
    # x packed on host as [128, QTC + 2*NPX]: cols [0, QTC) hold the QT
    # prototypes (col 32k+c = QT[128k+p, c], classes >= 19 zero); group
    # g's pixel block at cols [QTC + 2*base_g, ...), chunk k at
    # block-local cols [k*n, (k+1)*n)
    x_t = nc.dram_tensor("x", [128, QTC + 2 * NPX], EMB_DT,
                         kind="ExternalInput")
    # onehot u8 (group-blocked)
    meta_t = nc.dram_tensor("meta", [128, LBB_COLS], mybir.dt.uint8,
                            kind="ExternalInput")
    OUTC = 16  # pad to 64 B so the single output descriptor is aligned
    # acc cols: 0..NG-2 = s1 of g0..g6, NG-1..NG+1 = sqA/sqB/sqC,
    #           NG+2 = s1 of g7, NG+3 = sqD  (MM1 reduces 0..NG+1, MM2 rest)
    out_t = nc.dram_tensor("out", [1, OUTC], mybir.dt.float32,
                           kind="ExternalOutput")

    x = x_t.ap()
    AO = mybir.AluOpType

    with TileContext(nc) as tc:
        with (
            tc.tile_pool(name="xp", bufs=1) as xpool,
            tc.tile_pool(name="mp", bufs=1) as mpool,
            tc.tile_pool(name="scr", bufs=4) as spool,
            tc.tile_pool(name="jnk", bufs=2) as jpool,
            tc.tile_pool(name="acc", bufs=1) as apool,
            tc.tile_pool(name="ps", bufs=3, space="PSUM") as pspool,
            tc.tile_pool(name="rps", bufs=1, space="PSUM") as rpool,
        ):
            # single sync-ring stream: x0 (with qt at its head), onehot,
            # then the remaining x tiles
            xt = {}
            metat = None
            base = 0
            for g, n in enumerate(XTILES):
                ext = QTC if g == 0 else 0
                t = xpool.tile([128, ext + 2 * n], EMB_DT, tag=f"xg{g}")
                nc.sync.dma_start(
                    t[:, :],
                    x[:, QTC + 2 * base - ext:QTC + 2 * base + 2 * n])
                xt[g] = t
                if g == 0:
                    metat = mpool.tile([128, LBB_COLS], mybir.dt.uint8)
                    nc.sync.dma_start(metat[:, :], meta_t.ap())
                base += n

            lbbt = metat[:, :]
            qt_sb = xt[0][:, 0:2 * CP]

            acc = apool.tile([128, NG + 4], mybir.dt.float32)
            rps = rpool.tile([128, NG + 4], mybir.dt.float32, tag="rps")
            t1all = mpool.tile([128, LBB_COLS], mybir.dt.bfloat16)

            for g, n in enumerate(XTILES):
                cg = CGS[g]
                off = OFFS[g]
                ext = QTC if g == 0 else 0
                ps = pspool.tile([128, cg], mybir.dt.float32, tag="ps")
                for s in range(4):
                    for (mo, fb) in _blocks(cg):
                        for k in range(2):
                            col = ext + k * n + s * cg + mo
                            nc.tensor.matmul(
                                out=ps[CP * s:CP * (s + 1), mo:mo + fb],
                                lhsT=qt_sb[:, k * CP:(k + 1) * CP],
                                rhs=xt[g][:, col:col + fb],
                                start=(k == 0), stop=(k == 1),
                                tile_position=(0, CP * s))

                # t1 = onehot * sim into the resident buffer;
                # acc s1 col = row-sum(t1).  S2 squares are deferred and
                # batched (t1 is SBUF-resident), so ScalarE's big squares
                # run in mid-stream slack and never clog the drain.
                s1col = NG + 2 if g == NG - 1 else g
                nc.vector.scalar_tensor_tensor(
                    out=t1all[:, off:off + cg], in0=lbbt[:, off:off + cg],
                    scalar=1.0, in1=ps[:, :], op0=AO.mult, op1=AO.mult,
                    accum_out=acc[:, s1col:s1col + 1])
                for ci, (ga, gb) in enumerate(SQCH):
                    if g == gb - 1:
                        a, bcol = OFFS[ga], OFFS[gb]
                        jk = jpool.tile([128, bcol - a], mybir.dt.bfloat16,
                                        tag=f"jk{ci}")
                        if ci < len(SQCH) - 1:
                            # mid-stream square batches on ScalarE
                            nc.scalar.activation(
                                jk[:, :], t1all[:, a:bcol],
                                mybir.ActivationFunctionType.Square,
                                accum_out=acc[:, NG - 1 + ci:NG + ci])
                        else:
                            # small final chunk on the DVE (cheap acc read)
                            nc.vector.scalar_tensor_tensor(
                                out=jk[:, :], in0=t1all[:, a:bcol],
                                scalar=1.0, in1=t1all[:, a:bcol],
                                op0=AO.mult, op1=AO.mult,
                                accum_out=acc[:, NG + 3:NG + 4])
                if g == NG - 2:
                    # reduce everything except the last group's s1 and the
                    # final square chunk -- off the critical path
                    ones = nc.const_aps.aps[(mybir.dt.float32, 1.0)]
                    nc.tensor.matmul(
                        out=rps[0:1, 0:NG + 2], lhsT=ones,
                        rhs=acc[:, 0:NG + 2], start=True, stop=True,
                        tile_position=(0, 0))

            # fold the last group's pair into the same PSUM tile, then one
            # single-descriptor [1, 16] DMA
            ones = nc.const_aps.aps[(mybir.dt.float32, 1.0)]
            nc.tensor.matmul(out=rps[0:1, NG + 2:NG + 4], lhsT=ones,
                             rhs=acc[:, NG + 2:NG + 4], start=True,
                             stop=True, tile_position=(0, 0))
            res = apool.tile([1, 16], mybir.dt.float32)
            nc.vector.tensor_copy(res[:, 0:NG + 4], rps[0:1, :])
            nc.vector.memset(res[:, NG + 4:16], 0.0)
            nc.sync.dma_start(out_t.ap(), res[:, :])

    nc.compile()
    _CACHE["nc"] = nc
    return nc


def _prep_in_maps(emb, lb, segment_queue):
    emb = np.asarray(emb)
    lb = np.asarray(lb)
    q = np.asarray(segment_queue, dtype=np.float32)

    qt = np.zeros((D, CP), np.float32)
    qt[:, :C] = q.T
    # pack [2,128,CP] -> [128, 2*CP]: col 32k+c = QT[128k+p, c]
    qt = np.ascontiguousarray(
        qt.reshape(2, 128, CP).transpose(1, 0, 2).reshape(128, 2 * CP)
        .astype(EMB_NP))

    cls_pat = np.where(np.arange(CP) < C, np.arange(CP), -1)  # [32]

    in_maps = []
    for b in range(B):
        x8 = emb[b].reshape(2, 128, NPX).astype(EMB_NP)
        # pack per DMA tile: xb[p, QTC + 2*base + k*n + j] = x8[k, p, base+j]
        xb = np.empty((128, QTC + 2 * NPX), EMB_NP)
        xb[:, :2 * CP] = qt
        xb[:, 2 * CP:QTC] = 0
        base = 0
        for n in XTILES:
            blk = x8[:, :, base:base + n]            # [2, 128, n]
            xb[:, QTC + 2 * base:QTC + 2 * base + 2 * n] = (
                blk.transpose(1, 0, 2).reshape(128, 2 * n))
            base += n
        lbf = lb[b].reshape(-1).astype(np.float32)
        # onehot[32*s + c, off_g + j] = (lb[base_g + s*cg + j] == c)
        segs = []
        base = 0
        for g, n in enumerate(XTILES):
            cg = CGS[g]
            seg = lbf[base:base + n].reshape(4, 1, cg)
            segs.append((seg == cls_pat[None, :, None]).reshape(128, cg))
            base += n
        meta = np.ascontiguousarray(
            np.concatenate(segs, axis=1).astype(np.uint8))
        in_maps.append({
            "x": xb,
            "meta": meta,
        })
    return in_maps


def _reduce_outputs(results, count):
    s1 = 0.0
    s2 = 0.0
    for r in results:
        o = np.asarray(r["out"], dtype=np.float64)
        s1 += o[0, 0:NG - 1].sum() + o[0, NG + 2]
        s2 += o[0, NG - 1] + o[0, NG] + o[0, NG + 1] + o[0, NG + 3]
    num = count - 2.0 * s1 + s2
    return np.float32(num / count)


def run_on_cores(inputs, **kwargs):
    """Run the bass kernel on cores 0-7; returns (loss, BassKernelResults).

    The device occasionally reports a transient NRT_EXEC_UNIT_UNRECOVERABLE
    on a run that succeeds on immediate retry; retry a couple of times.
    """
    nc = _build()
    in_maps = _prep_in_maps(**inputs)
    count = float(np.count_nonzero(np.asarray(inputs["lb"]) != IGNORE))
    last_err = None
    for _ in range(3):
        try:
            res = bass_utils.run_bass_kernel_spmd(
                nc, in_maps, core_ids=list(range(NCORES)), **kwargs)
            return _reduce_outputs(res.results, count), res
        except Exception as e:  # transient device wedge -> retry
            last_err = e
    raise last_err


def kernel(emb, lb, segment_queue):
    loss, _ = run_on_cores({"emb": emb, "lb": lb, "segment_queue": segment_queue})
    return loss
